# revision 34
# baseline (speedup 1.0000x reference)
"""Diagonalizable linear plant (modal state-space scan) on 8 Trainium2 cores.

y[b,t] = Cz @ z[b,t-1] + D @ u[b,t],  z[b,t] = lam * z[b,t-1] + Bz @ u[b,t]
with z[b,-1] = z0[b] = x0[b] @ Q, Bz = Q^T Bmat, Cz = C Q.

Sharding: data-parallel over batch (16 batches -> 2 per core).

Block-8 formulation (the DVE scan instruction runs at ~2 cycles/element,
so the time axis is decimated 8x before it reaches the scan; everything
else is full 128x128xN=512 bf16 matmuls, fp32 PSUM):
  host packs u as uT8[(i*32+u), k] = u[8k+i, u]        (256 rows = 2 K-groups)
  PE   V_h = W2^T @ U          W2[(i,u),n] = lam_n^(7-i) Bz[n,u]
  DVE  zB = scan(lam^8, V)     block-boundary states z_{8k+7}
  PE   Y_g = WC^T @ zBprev + WU^T @ U     (g indexes (j,y) output groups)
       WC[n,(j,y)] = lam_n^j Cz[y,n]
       WU[(i,u),(j,y)] = (Cz lam^(j-1-i) Bz)[y,u] for i<j, D[y,u] for i=j, else 0
  host unpacks yT8[(32j+y), k] -> y[8k+j, y]
"""

import numpy as np

B, T, NX, NU, NY = 16, 8192, 256, 32, 32
NCORES = 8
BPC = B // NCORES   # batches per core
MB = 8              # time-block folded into matmul K
KCOL = T // MB      # block columns per batch (1024)
L = 512             # block-columns per chunk
NCHUNK = KCOL // L  # chunks per batch (2)

_PROG = None  # built Bass program, cached across kernel() calls


def _patch_tile_drain():
    """walrus codegen in this container rejects >1 sync wait on one SP
    TPB_CTRL instruction (terminal TileContext drain / NoOp). Split the
    drain's waits across preceding SP nops carrying one wait each."""
    import concourse.tile as tile
    import concourse.mybir as mybir
    from concourse.vector_clock import ScopedClock

    if getattr(tile.TileContext, "_drain_patched", False):
        return

    def _drain_and_barrier(self, tick_clock, wait_clock):
        nc = self.nc
        scratch = nc.sync.nop()
        wait_clock.add_sem_waits(
            scratch.ins, ScopedClock({None: tick_clock.global_clock})
        )
        si = scratch.ins.sync_info
        waits = list(si.on_wait) if si is not None else []
        scratch.ins.sync_info = mybir.SyncInfo(on_wait=waits[:1], on_update=[])
        for w in waits[1:]:
            n2 = nc.sync.nop()
            n2.ins.sync_info = mybir.SyncInfo(on_wait=[w], on_update=[])
        nc.sync.drain()
        nc.all_engine_barrier()
        assert self.sems is not None
        popped = nc._tile_sem_poison_stack.pop()
        assert popped is self._sem_poison
        nc.clear_and_free_semaphores(list(self.sems.allocated().values()))
        nc.all_engine_barrier()

    tile.TileContext._drain_and_barrier = _drain_and_barrier
    tile.TileContext._drain_patched = True


def _split_multi_waits(nc, mybir):
    """This container's walrus codegen accepts at most ONE sync wait per
    instruction. Hoist extra waits into standalone EventSemaphore nops on
    the same engine, placed immediately before the instruction."""
    ctr = [0]

    def fresh(engine, wait):
        ctr[0] += 1
        ev = mybir.InstEventSemaphore(name=f"I-wsplit-{ctr[0]}", ins=[], outs=[])
        ev.engine = engine
        ev.sync_info = mybir.SyncInfo(on_wait=[wait], on_update=[])
        nc.register_instruction(ev)
        return ev

    for fn in nc.m.functions:
        for bb in fn.blocks:
            out = []
            changed = False
            for inst in bb.instructions:
                si = inst.sync_info
                waits = list(si.on_wait) if si is not None else []
                if len(waits) > 1:
                    changed = True
                    for w in waits[:-1]:
                        out.append(fresh(inst.engine, w))
                    inst.sync_info = mybir.SyncInfo(
                        on_wait=[waits[-1]], on_update=list(si.on_update)
                    )
                out.append(inst)
            if changed:
                bb.instructions = out


def build_program():
    import concourse.bass as bass
    import concourse.tile as tile
    import concourse.mybir as mybir
    from contextlib import ExitStack

    _patch_tile_drain()
    f32 = mybir.dt.float32
    bf = mybir.dt.bfloat16

    nc = bass.Bass()
    # uH2[ch, g, row, k]: 2 KB rows [b0 | b1] per chunk-half — each 256 KB
    # DMA needs only 128 descriptors, halving descriptor-generation time
    # so every chunk arrives with slack even in the slow clock state
    uH2 = nc.declare_dram_parameter("uH2", [2, 2, 128, 2 * L], bf, isOutput=False)
    # wAll cols 0:8 carry lam^8 (bf16 hi+lo pair, recombined on DVE) and
    # z0 (bf16); then W2 | WC | WU blocks. No separate tiny-param DMA.
    wAll = nc.declare_dram_parameter("wAll", [128, 8 + 11 * 128], bf, isOutput=False)
    yT8s = nc.declare_dram_parameter("yT8s", [BPC, 2, 256, L], bf, isOutput=True)

    with ExitStack() as ctx:
        tc = ctx.enter_context(tile.TileContext(nc))
        const = ctx.enter_context(tc.tile_pool(name="const", bufs=1))
        vps = ctx.enter_context(tc.tile_pool(name="vps", bufs=2, space="PSUM"))
        yps = ctx.enter_context(tc.tile_pool(name="yps", bufs=2, space="PSUM"))
        zpool = ctx.enter_context(tc.tile_pool(name="z", bufs=6))
        yout = ctx.enter_context(tc.tile_pool(name="yo", bufs=4))

        # DMA plan: scalar queue carries all g=0 (rows 0:128) U tiles,
        # sync all g=1, both in unit-consumption order; weights ride the
        # gpsimd software queue (W2 first, then WC, WU for the Y phase).
        W2t = const.tile([128, 8 + 512], bf)
        nc.gpsimd.dma_start(W2t[:], wAll[:, 0 : 8 + 512])
        WCt = const.tile([128, 512], bf)
        nc.gpsimd.dma_start(WCt[:], wAll[:, 520:1032])
        WUt = const.tile([128, 384], bf)
        nc.gpsimd.dma_start(WUt[:], wAll[:, 1032:1416])
        # recover f32 params from the bf16 head columns of the W2 DMA
        pzt = const.tile([128, 8], f32)
        nc.gpsimd.tensor_copy(pzt[:], W2t[:, 0:8])
        lam8hi = pzt[:, 0:2]
        lam8lo = pzt[:, 2:4]
        z0t = pzt[:, 4:8]
        # UG[g]: [128, 2048] = [b0ch0 | b1ch0 | b0ch1 | b1ch1]
        UG = [const.tile([128, 4 * L], bf, name=f"UG{g}") for g in range(2)]
        qeng = [nc.scalar, nc.sync]
        for ch in range(2):
            for g in range(2):
                qeng[g].dma_start(UG[g][:, 2 * ch * L : 2 * (ch + 1) * L],
                                  uH2[ch, g, :, :])

        # PE warm-up during the DMA fill window
        dummy = const.tile([128, L], bf)
        nc.vector.memset(dummy[:], 0.0)
        WP = vps.tile([128, L], f32, name="WP", tag="V0")
        for _ in range(3):
            nc.tensor.matmul(WP[:], lhsT=dummy[:, 0:128], rhs=dummy[:],
                             start=True, stop=True)

        # lam broadcast built on DVE during the DMA fill window; the f32
        # lam^8 is recovered from the bf16 hi+lo pair (hi*1 then +lo)
        ones = const.tile([128, L], f32)
        nc.vector.memset(ones[:], 1.0)
        lam_bc = const.tile([128, 2 * L], f32)
        for h in range(2):
            nc.vector.tensor_scalar_mul(
                lam_bc[:, h * L : (h + 1) * L], ones[:], lam8hi[:, h : h + 1]
            )
            nc.vector.tensor_scalar_add(
                lam_bc[:, h * L : (h + 1) * L],
                lam_bc[:, h * L : (h + 1) * L], lam8lo[:, h : h + 1]
            )

        def w2blk(i):
            return W2t[:, 8 + 128 * i : 8 + 128 * (i + 1)]

        def wcblk(i):
            return WCt[:, 128 * i : 128 * (i + 1)]

        # wAll W2 block order: [g0h0, g1h0, g0h1, g1h1] (h=0 pair first)
        W2 = [[w2blk(0), w2blk(2)], [w2blk(1), w2blk(3)]]      # [g][h]
        WC = [[wcblk(0), wcblk(1)], [wcblk(2), wcblk(3)]]      # [h][g]
        WU00 = WUt[:, 0:128]
        WU01 = WUt[:, 128:256]
        WU11 = WUt[:, 256:384]                                 # WU[1][0] == 0

        mult = mybir.AluOpType.mult
        add = mybir.AluOpType.add

        prev_z = [[None, None] for _ in range(BPC)]

        def emit_vscan(c, b):
            sl = slice((2 * c + b) * L, (2 * c + b + 1) * L)
            U = [UG[0][:, sl], UG[1][:, sl]]
            zext = [None, None]
            for h in range(2):
                V = vps.tile([128, L], f32, name=f"V{h}_{b}_{c}", tag=f"V{h}")
                Z = zpool.tile([128, L + 1], bf, name=f"Z{h}_{b}_{c}",
                               tag=f"Z{h}")
                carry = (z0t[:, 2 * b + h : 2 * b + h + 1] if c == 0
                         else prev_z[b][h][:, L : L + 1])
                nc.tensor.matmul(V[:], lhsT=W2[0][h], rhs=U[0],
                                 start=True, stop=False)
                nc.tensor.matmul(V[:], lhsT=W2[1][h], rhs=U[1],
                                 start=False, stop=True)
                nc.vector.tensor_tensor_scan(
                    Z[:, 1 : L + 1], lam_bc[:, h * L : (h + 1) * L], V[:],
                    carry, mult, add,
                )
                nc.gpsimd.tensor_copy(Z[:, 0:1], carry)
                zext[h] = Z
            prev_z[b] = zext
            return U, zext

        def emit_y(c, b, U, zext, last=False):
            # last unit: g1 first — its cast+store path serializes on
            # scalar, so give it the head start; g0 rides vector+sync.
            # Matmuls for the two PSUM groups are interleaved stage-by-
            # stage so only the final WC(h1) pair waits on the last scan.
            gorder = [1, 0] if last else [0, 1]
            Yt = {}
            chains = {}
            for g in gorder:
                Yt[g] = yps.tile([128, L], f32, name=f"Y{g}_{b}_{c}",
                                 tag=f"Y{g}")
                wu = ([(WU00, U[0])] if g == 0
                      else [(WU01, U[0]), (WU11, U[1])])
                wc = [(WC[0][g], zext[0][:, 0:L]), (WC[1][g], zext[1][:, 0:L])]
                # early units: WC weights arrive before WU; late units: WU
                # is long loaded and scan h1 lands last, so WU goes first
                chains[g] = wc + wu if c == 0 else wu + wc
            pos = {g: 0 for g in gorder}
            nstage = max(len(chains[g]) for g in gorder)
            for i in range(nstage):
                for g in gorder:
                    ch = chains[g]
                    # keep shorter chains back-loaded so both finish last
                    if len(ch) - pos[g] < nstage - i:
                        continue
                    lhsT, rhs = ch[pos[g]]
                    nc.tensor.matmul(Yt[g][:], lhsT=lhsT, rhs=rhs,
                                     start=(pos[g] == 0),
                                     stop=(pos[g] == len(ch) - 1))
                    pos[g] += 1
            for g in gorder:
                Y = Yt[g]
                Ysb = yout.tile([128, L], bf, name=f"Ysb{g}_{b}_{c}",
                                tag=f"Ysb{g}")
                oeng = nc.sync if g == 0 else nc.scalar
                dst = yT8s[b, c, 128 * g : 128 * (g + 1), :]
                if not last:
                    nc.scalar.copy(Ysb[:], Y[:])
                    oeng.dma_start(dst, Ysb[:])
                else:
                    # tail: halves; vector (free after the last scan) takes
                    # g0 plus g1's second half, scalar only g1's first —
                    # stores fan out to both queues
                    H = L // 2
                    for p in range(2):
                        s = slice(p * H, (p + 1) * H)
                        if g == 0 or p == 1:
                            nc.vector.tensor_copy(Ysb[:, s], Y[:, s])
                        else:
                            nc.scalar.copy(Ysb[:, s], Y[:, s])
                        oeng.dma_start(dst[:, s], Ysb[:, s])

        units = [(c, b) for c in range(NCHUNK) for b in range(BPC)]
        pending = []
        for (c, b) in units:
            U, zext = emit_vscan(c, b)
            pending.append((c, b, U, zext))
            if len(pending) > 2:
                emit_y(*pending.pop(0))
        for i, p in enumerate(pending):
            emit_y(*p, last=(i == len(pending) - 1))

    _split_multi_waits(nc, mybir)
    return nc


def _host_prep(x0, u, Q, lam, Bmat, C, D):
    import ml_dtypes

    f = np.float32
    bfd = ml_dtypes.bfloat16
    lam = lam.astype(f)
    Bz = (Q.T.astype(f) @ Bmat.astype(f)).astype(f)      # (NX, NU)
    Cz = (C.astype(f) @ Q.astype(f)).astype(f)           # (NY, NX)
    z0 = (x0.astype(f) @ Q.astype(f)).astype(f)          # (B, NX)

    lam_p = np.stack([lam**j for j in range(MB)])         # (MB, NX)

    # W2[(i*32+u), n] = lam_n^(MB-1-i) * Bz[n, u]
    W2 = np.einsum("in,nu->iun", lam_p[::-1], Bz).reshape(MB * NU, NX)
    # WC[n, (32j+y)] = lam_n^j * Cz[y, n]
    WC = np.einsum("jn,yn->njy", lam_p, Cz).reshape(NX, MB * NY)
    # WU[(i*32+u), (32j+y)]
    WU = np.zeros((MB * NU, MB * NY), dtype=f)
    for j in range(MB):
        for i in range(MB):
            if i < j:
                Mji = (Cz * lam_p[j - 1 - i][None, :]) @ Bz   # (NY, NU)
                WU[i * NU : (i + 1) * NU, j * NY : (j + 1) * NY] = Mji.T
            elif i == j:
                WU[i * NU : (i + 1) * NU, j * NY : (j + 1) * NY] = D.T.astype(f)

    blocks = []
    for h in range(2):          # W2 order [g0h0, g1h0, g0h1, g1h1]
        for g in range(2):
            blocks.append(W2[128 * g : 128 * (g + 1), 128 * h : 128 * (h + 1)])
    for h in range(2):          # WC[h][g]
        for g in range(2):
            blocks.append(WC[128 * h : 128 * (h + 1), 128 * g : 128 * (g + 1)])
    # WU[g2][g] blocks; WU[1][0] is identically zero (i > j) and skipped
    blocks.append(WU[0:128, 0:128])      # WU00
    blocks.append(WU[0:128, 128:256])    # WU01
    blocks.append(WU[128:256, 128:256])  # WU11
    wAll = np.concatenate(blocks, axis=1).astype(bfd)     # (128, 11*128)
    wAll = np.asarray(wAll)

    # uT8[b, (i*32+u), k] = u[b, 8k+i, u]
    uT8 = np.ascontiguousarray(
        u.reshape(B, KCOL, MB, NU).transpose(0, 2, 3, 1).reshape(B, MB * NU, KCOL)
    ).astype(bfd)

    lam8 = (lam.astype(np.float64) ** MB).astype(f)
    lam8c = np.stack([lam8[:128], lam8[128:]], axis=1).astype(f)  # (128, 2)
    lam_hi = lam8c.astype(bfd)                                    # bf16 hi
    lam_lo = (lam8c - lam_hi.astype(f)).astype(bfd)               # bf16 lo
    return wAll, z0, uT8, lam_hi, lam_lo


def make_in_maps(x0, u, Q, lam, Bmat, C, D):
    import ml_dtypes

    bfd = ml_dtypes.bfloat16
    wAll, z0, uT8, lam_hi, lam_lo = _host_prep(x0, u, Q, lam, Bmat, C, D)
    in_maps = []
    for cidx in range(NCORES):
        sl = slice(cidx * BPC, (cidx + 1) * BPC)
        z0_c = z0[sl]
        z0c = z0_c.reshape(BPC, 2, 128).transpose(2, 0, 1).reshape(128, 2 * BPC)
        wAll_c = np.ascontiguousarray(
            np.concatenate([lam_hi, lam_lo, z0c.astype(bfd), wAll], axis=1)
        )
        ut = uT8[sl].reshape(BPC, 2, 128, KCOL)  # (b, g, row, k)
        # uH2[ch, g, row, :] = [b0 chunk-ch | b1 chunk-ch]
        uH2 = np.ascontiguousarray(
            np.stack(
                [
                    np.concatenate(
                        [ut[0, :, :, ch * L : (ch + 1) * L],
                         ut[1, :, :, ch * L : (ch + 1) * L]],
                        axis=2,
                    )
                    for ch in range(2)
                ],
                axis=0,
            )
        )
        in_maps.append(
            {
                "uH2": uH2,
                "wAll": wAll_c,
            }
        )
    return in_maps


def kernel(x0, u, Q, lam, Bmat, C, D):
    global _PROG
    from concourse.bass_utils import run_bass_kernel_spmd

    if _PROG is None:
        _PROG = build_program()
    in_maps = make_in_maps(x0, u, Q, lam, Bmat, C, D)
    res = run_bass_kernel_spmd(_PROG, in_maps, list(range(NCORES)))
    y = np.empty((B, T, NY), dtype=np.float32)
    for cidx in range(NCORES):
        yT8s_c = res.results[cidx]["yT8s"].astype(np.float32)  # (BPC, 2, 256, L)
        # y[b, 8*(ch*L+k)+j, yy] = yT8s[b, ch, 32j+yy, k]
        y[cidx * BPC : (cidx + 1) * BPC] = (
            yT8s_c.reshape(BPC, 2, MB, NY, L)
            .transpose(0, 1, 4, 2, 3)
            .reshape(BPC, T, NY)
        )
    return y



# revision 36
# speedup vs baseline: 1.0014x; 1.0014x over previous
"""Diagonalizable linear plant (modal state-space scan) on 8 Trainium2 cores.

y[b,t] = Cz @ z[b,t-1] + D @ u[b,t],  z[b,t] = lam * z[b,t-1] + Bz @ u[b,t]
with z[b,-1] = z0[b] = x0[b] @ Q, Bz = Q^T Bmat, Cz = C Q.

Sharding: data-parallel over batch (16 batches -> 2 per core).

Block-8 formulation (the DVE scan instruction runs at ~2 cycles/element,
so the time axis is decimated 8x before it reaches the scan; everything
else is full 128x128xN=512 bf16 matmuls, fp32 PSUM):
  host packs u as uT8[(i*32+u), k] = u[8k+i, u]        (256 rows = 2 K-groups)
  PE   V_h = W2^T @ U          W2[(i,u),n] = lam_n^(7-i) Bz[n,u]
  DVE  zB = scan(lam^8, V)     block-boundary states z_{8k+7}
  PE   Y_g = WC^T @ zBprev + WU^T @ U     (g indexes (j,y) output groups)
       WC[n,(j,y)] = lam_n^j Cz[y,n]
       WU[(i,u),(j,y)] = (Cz lam^(j-1-i) Bz)[y,u] for i<j, D[y,u] for i=j, else 0
  host unpacks yT8[(32j+y), k] -> y[8k+j, y]
"""

import numpy as np

B, T, NX, NU, NY = 16, 8192, 256, 32, 32
NCORES = 8
BPC = B // NCORES   # batches per core
MB = 8              # time-block folded into matmul K
KCOL = T // MB      # block columns per batch (1024)
L = 512             # block-columns per chunk
NCHUNK = KCOL // L  # chunks per batch (2)

_PROG = None  # built Bass program, cached across kernel() calls


def _patch_tile_drain():
    """walrus codegen in this container rejects >1 sync wait on one SP
    TPB_CTRL instruction (terminal TileContext drain / NoOp). Split the
    drain's waits across preceding SP nops carrying one wait each."""
    import concourse.tile as tile
    import concourse.mybir as mybir
    from concourse.vector_clock import ScopedClock

    if getattr(tile.TileContext, "_drain_patched", False):
        return

    def _drain_and_barrier(self, tick_clock, wait_clock):
        nc = self.nc
        scratch = nc.sync.nop()
        wait_clock.add_sem_waits(
            scratch.ins, ScopedClock({None: tick_clock.global_clock})
        )
        si = scratch.ins.sync_info
        waits = list(si.on_wait) if si is not None else []
        scratch.ins.sync_info = mybir.SyncInfo(on_wait=waits[:1], on_update=[])
        for w in waits[1:]:
            n2 = nc.sync.nop()
            n2.ins.sync_info = mybir.SyncInfo(on_wait=[w], on_update=[])
        nc.sync.drain()
        nc.all_engine_barrier()
        assert self.sems is not None
        popped = nc._tile_sem_poison_stack.pop()
        assert popped is self._sem_poison
        nc.clear_and_free_semaphores(list(self.sems.allocated().values()))
        nc.all_engine_barrier()

    tile.TileContext._drain_and_barrier = _drain_and_barrier
    tile.TileContext._drain_patched = True


def _split_multi_waits(nc, mybir):
    """This container's walrus codegen accepts at most ONE sync wait per
    instruction. Hoist extra waits into standalone EventSemaphore nops on
    the same engine, placed immediately before the instruction."""
    ctr = [0]

    def fresh(engine, wait):
        ctr[0] += 1
        ev = mybir.InstEventSemaphore(name=f"I-wsplit-{ctr[0]}", ins=[], outs=[])
        ev.engine = engine
        ev.sync_info = mybir.SyncInfo(on_wait=[wait], on_update=[])
        nc.register_instruction(ev)
        return ev

    for fn in nc.m.functions:
        for bb in fn.blocks:
            out = []
            changed = False
            for inst in bb.instructions:
                si = inst.sync_info
                waits = list(si.on_wait) if si is not None else []
                if len(waits) > 1:
                    changed = True
                    for w in waits[:-1]:
                        out.append(fresh(inst.engine, w))
                    inst.sync_info = mybir.SyncInfo(
                        on_wait=[waits[-1]], on_update=list(si.on_update)
                    )
                out.append(inst)
            if changed:
                bb.instructions = out


def build_program():
    import concourse.bass as bass
    import concourse.tile as tile
    import concourse.mybir as mybir
    from contextlib import ExitStack

    _patch_tile_drain()
    f32 = mybir.dt.float32
    bf = mybir.dt.bfloat16

    nc = bass.Bass()
    # uH2[ch, g, row, k]: 2 KB rows [b0 | b1] per chunk-half — each 256 KB
    # DMA needs only 128 descriptors, halving descriptor-generation time
    # so every chunk arrives with slack even in the slow clock state
    uH2 = nc.declare_dram_parameter("uH2", [2, 2, 128, 2 * L], bf, isOutput=False)
    # wAll cols 0:8 carry lam^8 (bf16 hi+lo pair, recombined on DVE) and
    # z0 (bf16); then W2 | WC | WU blocks. No separate tiny-param DMA.
    wAll = nc.declare_dram_parameter("wAll", [128, 8 + 11 * 128], bf, isOutput=False)
    yT8s = nc.declare_dram_parameter("yT8s", [BPC, 2, 256, L], bf, isOutput=True)

    with ExitStack() as ctx:
        tc = ctx.enter_context(tile.TileContext(nc))
        const = ctx.enter_context(tc.tile_pool(name="const", bufs=1))
        vps = ctx.enter_context(tc.tile_pool(name="vps", bufs=2, space="PSUM"))
        yps = ctx.enter_context(tc.tile_pool(name="yps", bufs=2, space="PSUM"))
        zpool = ctx.enter_context(tc.tile_pool(name="z", bufs=6))
        yout = ctx.enter_context(tc.tile_pool(name="yo", bufs=4))

        # DMA plan: scalar queue carries all g=0 (rows 0:128) U tiles,
        # sync all g=1, both in unit-consumption order; weights ride the
        # gpsimd software queue (W2 first, then WC, WU for the Y phase).
        W2t = const.tile([128, 8 + 512], bf)
        nc.gpsimd.dma_start(W2t[:], wAll[:, 0 : 8 + 512])
        WCt = const.tile([128, 512], bf)
        nc.gpsimd.dma_start(WCt[:], wAll[:, 520:1032])
        WUt = const.tile([128, 384], bf)
        nc.gpsimd.dma_start(WUt[:], wAll[:, 1032:1416])
        # recover f32 params from the bf16 head columns of the W2 DMA
        pzt = const.tile([128, 8], f32)
        nc.gpsimd.tensor_copy(pzt[:], W2t[:, 0:8])
        lam8hi = pzt[:, 0:2]
        lam8lo = pzt[:, 2:4]
        z0t = pzt[:, 4:8]
        # UG[g]: [128, 2048] = [b0ch0 | b1ch0 | b0ch1 | b1ch1]
        UG = [const.tile([128, 4 * L], bf, name=f"UG{g}") for g in range(2)]
        qeng = [nc.scalar, nc.sync]
        for ch in range(2):
            for g in range(2):
                qeng[g].dma_start(UG[g][:, 2 * ch * L : 2 * (ch + 1) * L],
                                  uH2[ch, g, :, :])

        # PE warm-up during the DMA fill window
        dummy = const.tile([128, L], bf)
        nc.vector.memset(dummy[:], 0.0)
        WP = vps.tile([128, L], f32, name="WP", tag="V0")
        for _ in range(3):
            nc.tensor.matmul(WP[:], lhsT=dummy[:, 0:128], rhs=dummy[:],
                             start=True, stop=True)

        # lam broadcast built on DVE during the DMA fill window; the f32
        # lam^8 is recovered from the bf16 hi+lo pair (hi*1 then +lo)
        ones = const.tile([128, L], f32)
        nc.vector.memset(ones[:], 1.0)
        lam_bc = const.tile([128, 2 * L], f32)
        for h in range(2):
            nc.vector.tensor_scalar_mul(
                lam_bc[:, h * L : (h + 1) * L], ones[:], lam8hi[:, h : h + 1]
            )
            nc.vector.tensor_scalar_add(
                lam_bc[:, h * L : (h + 1) * L],
                lam_bc[:, h * L : (h + 1) * L], lam8lo[:, h : h + 1]
            )

        def w2blk(i):
            return W2t[:, 8 + 128 * i : 8 + 128 * (i + 1)]

        def wcblk(i):
            return WCt[:, 128 * i : 128 * (i + 1)]

        # wAll W2 block order: [g0h0, g1h0, g0h1, g1h1] (h=0 pair first)
        W2 = [[w2blk(0), w2blk(2)], [w2blk(1), w2blk(3)]]      # [g][h]
        WC = [[wcblk(0), wcblk(1)], [wcblk(2), wcblk(3)]]      # [h][g]
        WU00 = WUt[:, 0:128]
        WU01 = WUt[:, 128:256]
        WU11 = WUt[:, 256:384]                                 # WU[1][0] == 0

        mult = mybir.AluOpType.mult
        add = mybir.AluOpType.add

        prev_z = [[None, None] for _ in range(BPC)]

        def emit_vscan(c, b):
            sl = slice((2 * c + b) * L, (2 * c + b + 1) * L)
            U = [UG[0][:, sl], UG[1][:, sl]]
            zext = [None, None]
            for h in range(2):
                V = vps.tile([128, L], f32, name=f"V{h}_{b}_{c}", tag=f"V{h}")
                Z = zpool.tile([128, L + 1], bf, name=f"Z{h}_{b}_{c}",
                               tag=f"Z{h}")
                carry = (z0t[:, 2 * b + h : 2 * b + h + 1] if c == 0
                         else prev_z[b][h][:, L : L + 1])
                nc.tensor.matmul(V[:], lhsT=W2[0][h], rhs=U[0],
                                 start=True, stop=False)
                nc.tensor.matmul(V[:], lhsT=W2[1][h], rhs=U[1],
                                 start=False, stop=True)
                nc.vector.tensor_tensor_scan(
                    Z[:, 1 : L + 1], lam_bc[:, h * L : (h + 1) * L], V[:],
                    carry, mult, add,
                )
                nc.gpsimd.tensor_copy(Z[:, 0:1], carry)
                zext[h] = Z
            prev_z[b] = zext
            return U, zext

        def emit_y(c, b, U, zext, last=False):
            # last unit: g1 first — its cast+store path serializes on
            # scalar, so give it the head start; g0 rides vector+sync.
            # Matmuls for the two PSUM groups are interleaved stage-by-
            # stage so only the final WC(h1) pair waits on the last scan.
            gorder = [1, 0] if last else [0, 1]
            Yt = {}
            chains = {}
            for g in gorder:
                Yt[g] = yps.tile([128, L], f32, name=f"Y{g}_{b}_{c}",
                                 tag=f"Y{g}")
                wu = ([(WU00, U[0])] if g == 0
                      else [(WU01, U[0]), (WU11, U[1])])
                wc = [(WC[0][g], zext[0][:, 0:L]), (WC[1][g], zext[1][:, 0:L])]
                # early units: WC weights arrive before WU; late units: WU
                # is long loaded and scan h1 lands last, so WU goes first
                chains[g] = wc + wu if c == 0 else wu + wc
            pos = {g: 0 for g in gorder}
            nstage = max(len(chains[g]) for g in gorder)
            for i in range(nstage):
                for g in gorder:
                    ch = chains[g]
                    # keep shorter chains back-loaded so both finish last
                    if len(ch) - pos[g] < nstage - i:
                        continue
                    lhsT, rhs = ch[pos[g]]
                    nc.tensor.matmul(Yt[g][:], lhsT=lhsT, rhs=rhs,
                                     start=(pos[g] == 0),
                                     stop=(pos[g] == len(ch) - 1))
                    pos[g] += 1
            for g in gorder:
                Y = Yt[g]
                Ysb = yout.tile([128, L], bf, name=f"Ysb{g}_{b}_{c}",
                                tag=f"Ysb{g}")
                oeng = nc.sync if g == 0 else nc.scalar
                dst = yT8s[b, c, 128 * g : 128 * (g + 1), :]
                if not last:
                    nc.scalar.copy(Ysb[:], Y[:])
                    oeng.dma_start(dst, Ysb[:])
                else:
                    # tail: halves; vector (free after the last scan) takes
                    # g0 plus g1's second half, scalar only g1's first —
                    # stores fan out to both queues
                    H = L // 2
                    for p in range(2):
                        s = slice(p * H, (p + 1) * H)
                        if g == 0 or p == 1:
                            nc.vector.tensor_copy(Ysb[:, s], Y[:, s])
                        else:
                            nc.scalar.copy(Ysb[:, s], Y[:, s])
                        oeng.dma_start(dst[:, s], Ysb[:, s])

        units = [(c, b) for c in range(NCHUNK) for b in range(BPC)]
        pending = []
        for (c, b) in units:
            U, zext = emit_vscan(c, b)
            pending.append((c, b, U, zext))
            if len(pending) > 2:
                emit_y(*pending.pop(0))
        for i, p in enumerate(pending):
            emit_y(*p, last=(i == len(pending) - 1))

    _split_multi_waits(nc, mybir)
    return nc


def _host_prep(x0, u, Q, lam, Bmat, C, D):
    import ml_dtypes

    f = np.float32
    bfd = ml_dtypes.bfloat16
    lam = lam.astype(f)
    Bz = (Q.T.astype(f) @ Bmat.astype(f)).astype(f)      # (NX, NU)
    Cz = (C.astype(f) @ Q.astype(f)).astype(f)           # (NY, NX)
    z0 = (x0.astype(f) @ Q.astype(f)).astype(f)          # (B, NX)

    lam_p = np.stack([lam**j for j in range(MB)])         # (MB, NX)

    # W2[(i*32+u), n] = lam_n^(MB-1-i) * Bz[n, u]
    W2 = np.einsum("in,nu->iun", lam_p[::-1], Bz).reshape(MB * NU, NX)
    # WC[n, (32j+y)] = lam_n^j * Cz[y, n]
    WC = np.einsum("jn,yn->njy", lam_p, Cz).reshape(NX, MB * NY)
    # WU[(i*32+u), (32j+y)]
    WU = np.zeros((MB * NU, MB * NY), dtype=f)
    for j in range(MB):
        for i in range(MB):
            if i < j:
                Mji = (Cz * lam_p[j - 1 - i][None, :]) @ Bz   # (NY, NU)
                WU[i * NU : (i + 1) * NU, j * NY : (j + 1) * NY] = Mji.T
            elif i == j:
                WU[i * NU : (i + 1) * NU, j * NY : (j + 1) * NY] = D.T.astype(f)

    blocks = []
    for h in range(2):          # W2 order [g0h0, g1h0, g0h1, g1h1]
        for g in range(2):
            blocks.append(W2[128 * g : 128 * (g + 1), 128 * h : 128 * (h + 1)])
    for h in range(2):          # WC[h][g]
        for g in range(2):
            blocks.append(WC[128 * h : 128 * (h + 1), 128 * g : 128 * (g + 1)])
    # WU[g2][g] blocks; WU[1][0] is identically zero (i > j) and skipped
    blocks.append(WU[0:128, 0:128])      # WU00
    blocks.append(WU[0:128, 128:256])    # WU01
    blocks.append(WU[128:256, 128:256])  # WU11
    wAll = np.concatenate(blocks, axis=1).astype(bfd)     # (128, 11*128)
    wAll = np.asarray(wAll)

    # uT8[b, (i*32+u), k] = u[b, 8k+i, u]
    uT8 = np.ascontiguousarray(
        u.reshape(B, KCOL, MB, NU).transpose(0, 2, 3, 1).reshape(B, MB * NU, KCOL)
    ).astype(bfd)

    lam8 = (lam.astype(np.float64) ** MB).astype(f)
    lam8c = np.stack([lam8[:128], lam8[128:]], axis=1).astype(f)  # (128, 2)
    lam_hi = lam8c.astype(bfd)                                    # bf16 hi
    lam_lo = (lam8c - lam_hi.astype(f)).astype(bfd)               # bf16 lo
    return wAll, z0, uT8, lam_hi, lam_lo


def make_in_maps(x0, u, Q, lam, Bmat, C, D):
    import ml_dtypes

    bfd = ml_dtypes.bfloat16
    wAll, z0, uT8, lam_hi, lam_lo = _host_prep(x0, u, Q, lam, Bmat, C, D)
    in_maps = []
    for cidx in range(NCORES):
        sl = slice(cidx * BPC, (cidx + 1) * BPC)
        z0_c = z0[sl]
        z0c = z0_c.reshape(BPC, 2, 128).transpose(2, 0, 1).reshape(128, 2 * BPC)
        wAll_c = np.ascontiguousarray(
            np.concatenate([lam_hi, lam_lo, z0c.astype(bfd), wAll], axis=1)
        )
        ut = uT8[sl].reshape(BPC, 2, 128, KCOL)  # (b, g, row, k)
        # uH2[ch, g, row, :] = [b0 chunk-ch | b1 chunk-ch]
        uH2 = np.ascontiguousarray(
            np.stack(
                [
                    np.concatenate(
                        [ut[0, :, :, ch * L : (ch + 1) * L],
                         ut[1, :, :, ch * L : (ch + 1) * L]],
                        axis=2,
                    )
                    for ch in range(2)
                ],
                axis=0,
            )
        )
        in_maps.append(
            {
                "uH2": uH2,
                "wAll": wAll_c,
            }
        )
    return in_maps


def kernel(x0, u, Q, lam, Bmat, C, D):
    global _PROG
    from concourse.bass_utils import run_bass_kernel_spmd

    if _PROG is None:
        _PROG = build_program()
    in_maps = make_in_maps(x0, u, Q, lam, Bmat, C, D)
    res = run_bass_kernel_spmd(_PROG, in_maps, list(range(NCORES)))
    y = np.empty((B, T, NY), dtype=np.float32)
    for cidx in range(NCORES):
        yT8s_c = res.results[cidx]["yT8s"].astype(np.float32)  # (BPC, 2, 256, L)
        # y[b, 8*(ch*L+k)+j, yy] = yT8s[b, ch, 32j+yy, k]
        y[cidx * BPC : (cidx + 1) * BPC] = (
            yT8s_c.reshape(BPC, 2, MB, NY, L)
            .transpose(0, 1, 4, 2, 3)
            .reshape(BPC, T, NY)
        )
    return y

